# revision 7
# baseline (speedup 1.0000x reference)
"""Trainium2 Bass kernel for nn_AIO_DownsampleCouplingBlock.

Reference computation (B=32, C=96, H=W=64, split 48/48):
  x1, x2 = x[:, :48], x[:, 48:]
  y2 = down(x2);  a1 = conv3x3_s2(x1, w_hi) + b_hi
  y2 = y2 * exp(2*tanh(0.2*a1[:192])) + a1[192:]
  y1 = down(x1);  a2 = conv3x3_s1(y2, w_lo) + b_lo
  y1 = y1 * exp(2*tanh(0.2*a2[:192])) + a2[192:]
  out = perm_w @ (concat(y1, y2) * scale + offset)   (channel matmul)
  scale = 0.2*softplus(0.5*act_norm), offset = act_offset

Strategy: data-parallel over batch, 4 samples per core on 8 cores.
All actnorm folding and channel reordering done host-side in numpy;
convs and the permutation run as bf16 matmuls on TensorE; tanh/exp on
ScalarE (with ln(scale) folded into the exp bias); affine mul on GPSIMD,
affine add (reading conv-t terms directly from PSUM) on VectorE.

Layouts (per sample, per core):
  x1hp [112, 33, 66] bf16: half-row-parity split of x1 with conv padding:
      partitions 0-47 = even x-rows (di=0), 64-111 = odd rows (di=1),
      rows 48-63 zero.  free = (i_pad, w_pad), x-row = 2*(i_pad-1)+di,
      w = w_pad-1, borders zero.
  x1h/x2h [96, 32, 64] bf16: partition di*48+c holds x rows di::2.
  y2 tiles A(dj=0)/B(dj=1) [96, 34, 34] bf16, zero borders, partition di*48+c.
  y1 tiles A/B [96, 32, 32] bf16.
  conv outputs in PSUM, 384 rows reordered as
      [E_dj0(96), T_dj0(0:32) | T_dj0(32:96), E_dj1(0:64) | E_dj1(64:96), T_dj1(96)]
  so every PSUM->SBUF piece starts at partition 0/32/64/96 (HW requirement).
"""
import sys, os
sys.path.insert(0, '/opt/trn_rl_repo')
import numpy as np
import ml_dtypes

import concourse.bass as bass
import concourse.mybir as mybir
from concourse.tile import TileContext
from concourse.bass_utils import run_bass_kernel_spmd

F32 = mybir.dt.float32
BF16 = mybir.dt.bfloat16
AF = mybir.ActivationFunctionType
MUL = mybir.AluOpType.mult
ADD = mybir.AluOpType.add

N_CORES = 8
B, C, H, W = 32, 96, 64, 64
S1 = 48                       # split_len1 == split_len2 == 48
SPS = B // N_CORES            # samples per core = 4
OH = OW = 32                  # output spatial
NCHUNK = 512                  # matmul free size (16 rows x 32 cols)

# ---------------------------------------------------------------------------
# channel-order helpers
# ---------------------------------------------------------------------------
# stored partition p (0..95) of a (dj) tile -> down-channel k = 4c + 2di + dj
def _down_idx(p, dj):
    di, c = divmod(p, 48)
    return 4 * c + 2 * di + dj

# PSUM row order for a conv output (384 rows).  Entries are reference
# channel indices into a1/a2 (0..191 = s-half, 192..383 = t-half).
def _psum_rows():
    e0 = [_down_idx(p, 0) for p in range(96)]
    e1 = [_down_idx(p, 1) for p in range(96)]
    t0 = [192 + _down_idx(p, 0) for p in range(96)]
    t1 = [192 + _down_idx(p, 1) for p in range(96)]
    return np.array(e0 + t0[:32] + t0[32:] + e1[:64] + t1 + e1[64:], np.int64)

_ROWS = _psum_rows()          # same structure for a1 and a2

def _bf(a):
    return np.ascontiguousarray(a).astype(ml_dtypes.bfloat16)


# ---------------------------------------------------------------------------
# host-side preprocessing
# ---------------------------------------------------------------------------
def _prepare(x, w_hi, b_hi, w_lo, b_lo, act_norm, act_offset, perm_w):
    x = np.asarray(x, np.float32)
    w_hi = np.asarray(w_hi, np.float32); w_lo = np.asarray(w_lo, np.float32)
    b_hi = np.asarray(b_hi, np.float32); b_lo = np.asarray(b_lo, np.float32)
    act_norm = np.asarray(act_norm, np.float32).reshape(-1)
    act_offset = np.asarray(act_offset, np.float32).reshape(-1)
    perm_w = np.asarray(perm_w, np.float32)

    scale = 0.2 * np.log1p(np.exp(0.5 * act_norm))          # softplus, beta=0.5
    offset = act_offset
    assert np.allclose(b_hi, 0) and np.allclose(b_lo, 0) and np.allclose(offset, 0), \
        "nonzero conv bias / actnorm offset not implemented"
    scale1, scale2 = scale[:192], scale[192:]

    # ---- input layouts (bf16) ----
    #   x1h/x2h: [B, 96, 32, 64], partition di*48+c <- x[:, base+c, di::2, :]
    def halfrows(xc):                      # xc [B, 48, 64, 64]
        v = xc.reshape(B, 48, 32, 2, 64).transpose(0, 3, 1, 2, 4)  # B,di,c,i,w
        return v.reshape(B, 96, 32, 64)
    x1h = _bf(halfrows(x[:, :48]))
    x2h = _bf(halfrows(x[:, 48:]))
    x1hp = np.zeros((B, 112, 33, 66), np.float32)
    x1hp[:, 0:48, 1:33, 1:65] = x[:, :48, 0::2, :]      # di=0: i_pad holds x-row 2*(i_pad-1)
    x1hp[:, 64:112, 1:33, 1:65] = x[:, :48, 1::2, :]    # di=1: i_pad holds x-row 2*i_pad-1
    x1hp = _bf(x1hp)

    # ---- conv_hi weights: 6 taps x [112, 384] ----
    # merged tap (kj, ki=1&2): rows 0-47 = w[.,c,1,kj], rows 64-111 = w[.,c,2,kj]
    # lone   tap (kj, ki=0):   rows 64-111 = w[.,c,0,kj]
    w_hi_eff = w_hi.copy()                 # [384, 48, 3, 3]
    w_hi_eff[192:] *= scale2[:, None, None, None]
    w_hi_r = w_hi_eff[_ROWS]               # PSUM row order
    lhsT_hi = np.zeros((112, 6, 384), np.float32)
    for kj in range(3):
        lhsT_hi[0:48, 2*kj+0] = w_hi_r[:, :, 1, kj].T
        lhsT_hi[64:112, 2*kj+0] = w_hi_r[:, :, 2, kj].T
        lhsT_hi[64:112, 2*kj+1] = w_hi_r[:, :, 0, kj].T
    lhsT_hi = _bf(lhsT_hi.reshape(112, 6 * 384))

    # ---- conv_lo weights: 9 taps x 2 tiles x [96, 384] ----
    w_lo_eff = w_lo.copy()                 # [384, 192, 3, 3]
    w_lo_eff[192:] *= scale1[:, None, None, None]
    w_lo_eff = w_lo_eff / scale2[None, :, None, None]      # y2 stored pre-scaled
    w_lo_r = w_lo_eff[_ROWS]
    kin = np.empty((2, 96), np.int64)
    for dj in range(2):
        kin[dj] = [_down_idx(p, dj) for p in range(96)]
    lhsT_lo = np.zeros((96, 9, 2, 384), np.float32)
    for ki in range(3):
        for kj in range(3):
            for dj in range(2):
                lhsT_lo[:, 3*ki+kj, dj] = w_lo_r[:, kin[dj], ki, kj].T
    lhsT_lo = _bf(lhsT_lo.reshape(96, 18 * 384))

    # ---- permutation weights: 4 tiles x [96, 384] ----
    lhsT_pm = np.zeros((96, 4, 384), np.float32)
    for dj in range(2):
        lhsT_pm[:, 0+dj] = perm_w[:, kin[dj]].T            # y1 tiles: pre idx k
        lhsT_pm[:, 2+dj] = perm_w[:, 192 + kin[dj]].T      # y2 tiles: 192+k
    lhsT_pm = _bf(lhsT_pm.reshape(96, 4 * 384))

    # ---- exp biases ln(scale), per stored partition ----
    ebias = np.zeros((96, 4), np.float32)                  # [:,0/1]=a1 dj, [:,2/3]=a2 dj
    for dj in range(2):
        ebias[:, 0+dj] = np.log(scale2[kin[dj]])
        ebias[:, 2+dj] = np.log(scale1[kin[dj]])

    per_core = []
    for ci in range(N_CORES):
        sl = slice(ci * SPS, (ci + 1) * SPS)
        per_core.append(dict(
            x1hp=np.ascontiguousarray(x1hp[sl].reshape(SPS, 112, 33 * 66)),
            x1h=np.ascontiguousarray(x1h[sl].reshape(SPS, 96, 32 * 64)),
            x2h=np.ascontiguousarray(x2h[sl].reshape(SPS, 96, 32 * 64)),
            lhsT_hi=lhsT_hi, lhsT_lo=lhsT_lo, lhsT_pm=lhsT_pm, ebias=ebias,
        ))
    return per_core


# ---------------------------------------------------------------------------
# device kernel builder
# ---------------------------------------------------------------------------
def build_kernel():
    nc = bass.Bass()
    p_x1hp = nc.declare_dram_parameter("x1hp", [SPS, 112, 33 * 66], BF16, isOutput=False)
    p_x1h = nc.declare_dram_parameter("x1h", [SPS, 96, 32 * 64], BF16, isOutput=False)
    p_x2h = nc.declare_dram_parameter("x2h", [SPS, 96, 32 * 64], BF16, isOutput=False)
    p_whi = nc.declare_dram_parameter("lhsT_hi", [112, 6 * 384], BF16, isOutput=False)
    p_wlo = nc.declare_dram_parameter("lhsT_lo", [96, 18 * 384], BF16, isOutput=False)
    p_wpm = nc.declare_dram_parameter("lhsT_pm", [96, 4 * 384], BF16, isOutput=False)
    p_eb = nc.declare_dram_parameter("ebias", [96, 4], F32, isOutput=False)
    p_out = nc.declare_dram_parameter("out", [SPS, 384, 1024], F32, isOutput=True)

    with TileContext(nc) as tc:
        with (
            tc.tile_pool(name="wt", bufs=1) as wt,
            tc.tile_pool(name="xin", bufs=2) as xin,
            tc.tile_pool(name="ttile", bufs=2) as tpool,
            tc.tile_pool(name="etile", bufs=2) as epool,
            tc.tile_pool(name="tmp", bufs=4) as mpool,
            tc.tile_pool(name="ytile", bufs=1) as ypool,
            tc.tile_pool(name="ost", bufs=3) as opool,
            tc.tile_pool(name="ps", bufs=8, space="PSUM") as ps,
        ):
            whi = wt.tile([112, 6 * 384], BF16)
            wlo = wt.tile([96, 18 * 384], BF16)
            wpm = wt.tile([96, 4 * 384], BF16)
            eb = wt.tile([96, 4], F32)
            nc.sync.dma_start(out=whi, in_=p_whi[:])
            nc.sync.dma_start(out=wlo, in_=p_wlo[:])
            nc.sync.dma_start(out=wpm, in_=p_wpm[:])
            nc.sync.dma_start(out=eb, in_=p_eb[:])

            # persistent y2 tiles (2 slots x 2 dj) with zero borders
            y2t = [[ypool.tile([96, 34 * 34], BF16, tag=f"y2_{sl}_{dj}", name=f"y2_{sl}_{dj}")
                    for dj in range(2)] for sl in range(2)]
            for sl in range(2):
                for dj in range(2):
                    nc.gpsimd.memset(y2t[sl][dj][:, :], 0.0)

            for s in range(SPS):
                slot = s % 2
                x1hp = xin.tile([112, 33 * 66], BF16, tag="x1hp")
                x1h = xin.tile([96, 32 * 64], BF16, tag="x1h")
                x2h = xin.tile([96, 32 * 64], BF16, tag="x2h")
                nc.sync.dma_start(out=x1hp, in_=p_x1hp[s])
                nc.sync.dma_start(out=x1h, in_=p_x1h[s])
                nc.sync.dma_start(out=x2h, in_=p_x2h[s])

                x1hp_v = x1hp.rearrange("p (i w) -> p i w", i=33)
                x1h_v = x1h.rearrange("p (i w) -> p i w", i=32)
                x2h_v = x2h.rearrange("p (i w) -> p i w", i=32)
                y2v = [y2t[slot][dj].rearrange("p (i w) -> p i w", i=34)
                       for dj in range(2)]

                # ============ conv_hi -> a1 (PSUM), E1, y2 ============
                a1 = [[ps.tile([128, NCHUNK], F32, tag="psum", name=f"a1_{s}_{m}_{n}") for n in range(2)]
                      for m in range(3)]
                for m in range(3):
                    for n in range(2):
                        first = True
                        for kj in range(3):
                            # merged tap ki=1&2: i_pad rows 16n+1 .. 16n+16
                            rhs = x1hp_v[0:112, 16*n+1:16*n+17, kj:kj+63:2]
                            nc.tensor.matmul(
                                a1[m][n][:, :], whi[0:112, (2*kj)*384 + 128*m:(2*kj)*384 + 128*m + 128],
                                rhs, start=first, stop=False)
                            first = False
                            # lone tap ki=0: i_pad rows 16n .. 16n+15
                            rhs = x1hp_v[0:112, 16*n:16*n+16, kj:kj+63:2]
                            nc.tensor.matmul(
                                a1[m][n][:, :], whi[0:112, (2*kj+1)*384 + 128*m:(2*kj+1)*384 + 128*m + 128],
                                rhs, start=False, stop=(kj == 2))

                self_affine(nc, tpool, epool, mpool, a1, eb, 0, x2h_v, y2v,
                            y2_mode=True)

                # ============ conv_lo -> a2 (PSUM), E2, y1 ============
                a2 = [[ps.tile([128, NCHUNK], F32, tag="psum", name=f"a2_{s}_{m}_{n}") for n in range(2)]
                      for m in range(3)]
                for m in range(3):
                    for n in range(2):
                        idx = 0
                        for ki in range(3):
                            for kj in range(3):
                                for dj in range(2):
                                    rhs = y2v[dj][0:96, 16*n+ki:16*n+ki+16, kj:kj+32]
                                    nc.tensor.matmul(
                                        a2[m][n][:, :],
                                        wlo[0:96, (6*ki+2*kj+dj)*384 + 128*m:(6*ki+2*kj+dj)*384 + 128*m + 128],
                                        rhs, start=(idx == 0), stop=(idx == 17))
                                    idx += 1

                y1t = [mpool.tile([96, 1024], BF16, tag=f"y1_{dj}", name=f"y1_{s}_{dj}") for dj in range(2)]
                y1v = [y1t[dj].rearrange("p (n f) -> p n f", n=2) for dj in range(2)]
                self_affine(nc, tpool, epool, mpool, a2, eb, 2, x1h_v, y1v,
                            y2_mode=False)

                # ============ permutation matmul + output ============
                ost = [opool.tile([128, 1024], F32, tag="ost", name=f"ost_{s}_{m}") for m in range(3)]
                for m in range(3):
                    for n in range(2):
                        po = ps.tile([128, NCHUNK], F32, tag="psum", name=f"po_{s}_{m}_{n}")
                        for t in range(2):     # y1 tiles
                            nc.tensor.matmul(
                                po[:, :], wpm[0:96, t*384 + 128*m:t*384 + 128*m + 128],
                                y1v[t][0:96, n, :], start=(t == 0), stop=False)
                        for t in range(2):     # y2 tiles (interior view)
                            rhs = y2v[t][0:96, 16*n+1:16*n+17, 1:33]
                            nc.tensor.matmul(
                                po[:, :], wpm[0:96, (2+t)*384 + 128*m:(2+t)*384 + 128*m + 128],
                                rhs, start=False, stop=(t == 1))
                        nc.vector.tensor_copy(ost[m][:, n*512:(n+1)*512], po[:, :])
                    nc.sync.dma_start(out=p_out[s, 128*m:128*m+128, :], in_=ost[m])
    return nc


def self_affine(nc, tpool, epool, mpool, a, eb, eb_base, xh_v, yv, y2_mode):
    """tanh/exp/mul/add chain: y[dj] = u[dj] * exp(2*tanh(0.2*a_s) + lnscale) + a_t.

    a: 3x2 PSUM tiles [128, 512] in _ROWS order.  xh_v: [96, 32, 64] view.
    yv: per-dj output views; y2_mode writes into padded [34,34] interior,
    else into [2, 512] flat chunks."""
    t = [tpool.tile([96, 1024], F32, tag=f"t{eb_base}_{dj}", name=f"t{eb_base}_{dj}") for dj in range(2)]
    e = [epool.tile([96, 1024], BF16, tag=f"e{eb_base}_{dj}", name=f"e{eb_base}_{dj}") for dj in range(2)]
    for n in range(2):
        ns = slice(n * 512, (n + 1) * 512)
        # tanh pieces (PSUM -> t tiles, f32)
        nc.scalar.activation(t[0][0:96, ns], a[0][n][0:96, :], AF.Tanh, scale=0.2)
        nc.scalar.activation(t[1][0:64, ns], a[1][n][64:128, :], AF.Tanh, scale=0.2)
        nc.scalar.activation(t[1][64:96, ns], a[2][n][96:128, :], AF.Tanh, scale=0.2)
        for dj in range(2):
            # exp with ln(scale) bias -> bf16 E
            nc.scalar.activation(e[dj][0:96, ns], t[dj][0:96, ns], AF.Exp,
                                 bias=eb[0:96, eb_base + dj:eb_base + dj + 1], scale=2.0)
            # mul: tmp = u_dj * E_dj   (gpsimd, all-SBUF bf16)
            tmp = mpool.tile([96, 512], BF16, tag=f"tmp{eb_base}_{dj}", name=f"tmp{eb_base}_{dj}")
            uview = xh_v[0:96, 16*n:16*n+16, dj:dj+63:2]
            ev = e[dj][0:96, ns].rearrange("p (i w) -> p i w", i=16)
            tmv = tmp.rearrange("p (i w) -> p i w", i=16)
            nc.gpsimd.tensor_tensor(out=tmv[0:96], in0=uview, in1=ev, op=MUL)
            # adds: y = tmp + T (T pieces from PSUM)
            if y2_mode:
                dst = yv[dj][0:96, 16*n+1:16*n+17, 1:33]
            else:
                dst = yv[dj][0:96, n, :].rearrange("p (i w) -> p i w", i=16)
            tv = tmp.rearrange("p (i w) -> p i w", i=16)
            if dj == 0:
                nc.vector.tensor_tensor(out=dst[0:32], in0=tv[0:32], in1=a[0][n][96:128, :].rearrange("p (i w) -> p i w", i=16), op=ADD)
                nc.vector.tensor_tensor(out=dst[32:64], in0=tv[32:64], in1=a[1][n][0:32, :].rearrange("p (i w) -> p i w", i=16), op=ADD)
                nc.vector.tensor_tensor(out=dst[64:96], in0=tv[64:96], in1=a[1][n][32:64, :].rearrange("p (i w) -> p i w", i=16), op=ADD)
            else:
                nc.vector.tensor_tensor(out=dst[0:96], in0=tv[0:96], in1=a[2][n][0:96, :].rearrange("p (i w) -> p i w", i=16), op=ADD)


# ---------------------------------------------------------------------------
# entry point
# ---------------------------------------------------------------------------
_CACHE = {}

def _get_nc():
    if "nc" not in _CACHE:
        nc = build_kernel()
        try:
            sys.path.insert(0, os.path.dirname(os.path.abspath(__file__)))
            import hwutil
            hwutil.install()
            hwutil.split_excess_waits(nc)
        except ImportError:
            _split_excess_waits_inline(nc)
        _CACHE["nc"] = nc
    return _CACHE["nc"]


def _split_excess_waits_inline(nc):
    import bass_rust
    n_fix = 0
    for fn in nc.m.functions:
        for blk in fn.blocks:
            insts = blk.instructions
            out = []
            for inst in insts:
                si = inst.sync_info
                cap = 2 if inst.opcode == "EventSemaphore" else 1
                if si is not None and len(si.on_wait) > cap:
                    waits = list(si.on_wait)
                    keep, extra = waits[:cap], waits[cap:]
                    for ci in range(0, len(extra), 2):
                        n_fix += 1
                        ev = mybir.InstEventSemaphore(name=f"I-waitfix-{n_fix}", ins=[], outs=[])
                        ev.engine = inst.engine
                        ev.sync_info = bass_rust.SyncInfo(on_wait=extra[ci:ci+2], on_update=[])
                        nc.register_instruction(ev, overwrite=True)
                        out.append(ev)
                    inst.sync_info = bass_rust.SyncInfo(on_wait=keep, on_update=list(si.on_update))
                out.append(inst)
            if len(out) != len(insts):
                blk.instructions = out
    return n_fix


def _install_profile_shim():
    import types
    name = "antenv.axon_hooks"
    if name in sys.modules:
        return
    try:
        from trn_agent_boot.trn_boot import _ntff_profile_via_ctypes
        hook = _ntff_profile_via_ctypes('/opt/axon/libaxon_pjrt.so')
    except Exception:
        hook = None
    mod = types.ModuleType(name)
    mod._hook = hook
    mod.get_axon_ntff_profile_hook = lambda: mod._hook
    mod.set_axon_ntff_profile_hook = lambda h: setattr(mod, '_hook', h)
    sys.modules[name] = mod


def run(inputs, trace=False):
    _install_profile_shim()
    per_core = _prepare(**inputs)
    nc = _get_nc()
    res = run_bass_kernel_spmd(nc, per_core, core_ids=list(range(N_CORES)), trace=trace)
    outs = [res.results[i]["out"].reshape(SPS, 384, 32, 32) for i in range(N_CORES)]
    full = np.concatenate(outs, axis=0).astype(np.float32)
    return full, res


def kernel(**inputs):
    full, _ = run(inputs, trace=False)
    return full
